# revision 18
# baseline (speedup 1.0000x reference)
"""GAT (2-layer graph-attention) Trainium2 kernel, SPMD over 8 NeuronCores.

Sharding: core c owns destination nodes i in [c*1024, (c+1)*1024).  It receives
its row-slab of adj pre-transposed on host (shape [8192 src, 1024 dst]) so the
source dimension lands on SBUF partitions, which is what the PE matmul
contracts over.  adj is binary, so the on-load f32->bf16 cast is exact.

Math: with s_ij = f1_i + f2_j,
    exp(leaky_relu(s, 0.2)) = e^{0.2 f1_i} e^{0.2 f2_j} * max(e^{0.8 f2_j} e^{0.8 f1_i}, 1)
The e^{0.2 f2_j} factor is folded into the fts matmul operand, the e^{0.2 f1_i}
factor cancels between numerator and softmax row-sum, so the per-element work
is one dual-op tensor_scalar (mult+max) and one masking tensor_tensor mult.
The matmul computes [fts*b' ; b'] so row-sums come out of the same accumulation.
Between the two attention layers only the scalar per-node projection g needs to
be exchanged: a 4KB-per-core AllGather.
"""

import os
import sys

sys.path.insert(0, "/opt/trn_rl_repo")

import numpy as np
import ml_dtypes

BF16 = ml_dtypes.bfloat16

N = 8192
NCORES = 8
IPC = N // NCORES  # 1024 destination nodes per core
HID = 64
HEADS = 8
JC = N // 128  # 64 source chunks of 128

LAST_RESULT = None  # BassKernelResults of the most recent launch (for test.py)
LAST_PROG = None  # Bass program of the most recent launch (for test.py bench)
LAST_IN_MAPS = None  # per-core input maps of the most recent launch


def _build_program(f1_wo, f1_bo, f2_wo, f2_bo):
    import concourse.bass as bass
    import concourse.bacc as bacc
    import concourse.mybir as mybir
    from concourse.tile import TileContext

    dt = mybir.dt
    op = mybir.AluOpType
    AF = mybir.ActivationFunctionType

    ts_engine = os.environ.get("GAT_TS_ENGINE", "gpsimd")

    # Bacc (not plain Bass): its compile passes split multi-semaphore waits
    # into event-semaphore chains, which the TT/TS instruction formats need.
    nc = bacc.Bacc(None, num_devices=NCORES)

    adjt_in = nc.dram_tensor("adjt", [N, IPC], dt.float32, kind="ExternalInput")
    fa2_in = nc.dram_tensor("fa2", [128, JC * 65], dt.bfloat16, kind="ExternalInput")
    ct_in = nc.dram_tensor("ctile", [128, JC], dt.float32, kind="ExternalInput")
    dr_in = nc.dram_tensor("drep", [128, IPC], dt.bfloat16, kind="ExternalInput")
    ws_in = nc.dram_tensor("wsum", [HID, 1], dt.float32, kind="ExternalInput")
    vb1_in = nc.dram_tensor("vb1t", [HID, IPC], dt.float32, kind="ExternalInput")
    vbo_in = nc.dram_tensor("vbot", [1, IPC], dt.float32, kind="ExternalInput")
    eye_in = nc.dram_tensor("eye", [64, 64], dt.float32, kind="ExternalInput")

    h_out = nc.dram_tensor("h_out", [HID, IPC], dt.float32, kind="ExternalOutput")
    lg_out = nc.dram_tensor("lg_out", [1, IPC], dt.float32, kind="ExternalOutput")

    ts_eng = {"gpsimd": nc.gpsimd, "vector": nc.vector}[ts_engine]

    with TileContext(nc) as tc:
        with (
            tc.tile_pool(name="big", bufs=1) as bigp,
            tc.tile_pool(name="sb", bufs=1) as sbp,
            tc.tile_pool(name="vw", bufs=3) as vwp,
            tc.tile_pool(name="ps", bufs=1, space="PSUM") as psp,
            tc.tile_pool(name="dram", bufs=1, space="DRAM") as drp,
        ):
            # ---- persistent SBUF state ----
            adjT = bigp.tile([128, JC * IPC], dt.bfloat16)  # 16 MB: adj^T, bf16
            fa2_sb = sbp.tile([128, JC * 65], dt.bfloat16)
            ct_sb = sbp.tile([128, JC], dt.float32)
            dr_sb = sbp.tile([128, IPC], dt.bfloat16)
            ws_sb = sbp.tile([HID, 1], dt.float32)
            vb1_sb = sbp.tile([HID, IPC], dt.float32)
            vbo_sb = sbp.tile([1, IPC], dt.float32)
            eye_sb = sbp.tile([64, 64], dt.float32)
            ones_sb = sbp.tile([1, 128], dt.float32)

            nc.sync.dma_start(out=fa2_sb[:], in_=fa2_in[:, :])
            nc.sync.dma_start(out=ct_sb[:], in_=ct_in[:, :])
            nc.sync.dma_start(out=dr_sb[:], in_=dr_in[:, :])
            nc.sync.dma_start(out=ws_sb[:], in_=ws_in[:, :])
            nc.sync.dma_start(out=vb1_sb[:], in_=vb1_in[:, :])
            nc.sync.dma_start(out=vbo_sb[:], in_=vbo_in[:, :])
            nc.sync.dma_start(out=eye_sb[:], in_=eye_in[:, :])
            nc.vector.memset(ones_sb[:], 1.0)

            # adj^T slab load: f32 HBM -> bf16 SBUF (SWDGE cast DMA), 8 x 4MB
            adjt_r = adjt_in[:, :].rearrange("(b q p) i -> b p q i", p=128, q=8)
            for b in range(8):
                dst = adjT[:, b * 8 * IPC : (b + 1) * 8 * IPC].rearrange(
                    "p (q i) -> p q i", q=8
                )
                nc.gpsimd.dma_start(out=dst, in_=adjt_r[b])

            # ---- layer 1 main loop ----
            # The TensorScalarPtr ISA slot only fits one sync wait, so absorb
            # the input-DMA dependencies into a multi-wait-capable op on the
            # same engine first.
            touch = sbp.tile([128, 1], dt.float32, tag="touch")
            nc.vector.tensor_copy(touch[:], dr_sb[:, 0:1])
            nc.vector.tensor_copy(touch[:], ct_sb[:, 0:1])

            ps_a = psp.tile([65, 512], dt.float32, tag="acc_a")
            ps_b = psp.tile([65, 512], dt.float32, tag="acc_b")
            for jc in range(JC):
                if jc % 8 == 0:
                    # absorb this adj chunk's (possibly split) DMA semaphores
                    nc.vector.tensor_copy(touch[:], adjT[:, jc * IPC : jc * IPC + 1])
                v = vwp.tile([128, IPC], dt.bfloat16, tag="v")
                ts_eng.tensor_scalar(
                    v[:], dr_sb[:], ct_sb[:, jc : jc + 1], 1.0, op.mult, op.max
                )
                w = vwp.tile([128, IPC], dt.bfloat16, tag="w")
                nc.vector.tensor_tensor(
                    w[:], v[:], adjT[:, jc * IPC : (jc + 1) * IPC], op.mult
                )
                lhsT = fa2_sb[:, jc * 65 : (jc + 1) * 65]
                nc.tensor.matmul(
                    ps_a[:], lhsT, w[:, 0:512], start=(jc == 0), stop=(jc == JC - 1)
                )
                nc.tensor.matmul(
                    ps_b[:], lhsT, w[:, 512:1024], start=(jc == 0), stop=(jc == JC - 1)
                )

            # ---- layer 1 epilogue: h = elu(vals/rowsum + vbias1) ----
            rc_a = sbp.tile([1, 512], dt.float32, tag="rc_a")
            rc_b = sbp.tile([1, 512], dt.float32, tag="rc_b")
            nc.vector.reciprocal(rc_a[:], ps_a[64:65, :])
            nc.vector.reciprocal(rc_b[:], ps_b[64:65, :])
            ps_ra = psp.tile([64, 512], dt.float32, tag="bc_a")
            ps_rb = psp.tile([64, 512], dt.float32, tag="bc_b")
            nc.tensor.matmul(ps_ra[:], ones_sb[:, 0:64], rc_a[:], start=True, stop=True)
            nc.tensor.matmul(ps_rb[:], ones_sb[:, 0:64], rc_b[:], start=True, stop=True)
            rcr = sbp.tile([64, IPC], dt.float32)
            nc.vector.tensor_copy(rcr[:, 0:512], ps_ra[:])
            nc.vector.tensor_copy(rcr[:, 512:1024], ps_rb[:])
            xx = sbp.tile([64, IPC], dt.float32)
            nc.vector.tensor_tensor(xx[:, 0:512], ps_a[0:64, :], rcr[:, 0:512], op.mult)
            nc.vector.tensor_tensor(xx[:, 512:1024], ps_b[0:64, :], rcr[:, 512:1024], op.mult)
            nc.vector.tensor_tensor(xx[:], xx[:], vb1_sb[:], op.add)
            r1 = sbp.tile([64, IPC], dt.float32)
            nc.vector.tensor_scalar_max(r1[:], xx[:], 0.0)
            mn = sbp.tile([64, IPC], dt.float32, tag="rcr")  # rcr is dead here
            nc.vector.tensor_scalar_min(mn[:], xx[:], 0.0)
            ex = sbp.tile([64, IPC], dt.float32, tag="xx")  # xx dead after r1/mn
            nc.scalar.activation(ex[:], mn[:], AF.Exp)
            h_sb = sbp.tile([64, IPC], dt.float32)
            nc.vector.tensor_tensor(h_sb[:], r1[:], ex[:], op.add)
            nc.vector.tensor_scalar_sub(h_sb[:], h_sb[:], 1.0)
            nc.sync.dma_start(out=h_out[:, :], in_=h_sb[:])

            # ---- g = wsum . h ; AllGather g across cores ----
            ps_g1 = psp.tile([1, 512], dt.float32, tag="g1")
            ps_g2 = psp.tile([1, 512], dt.float32, tag="g2")
            nc.tensor.matmul(ps_g1[:], ws_sb[:], h_sb[:, 0:512], start=True, stop=True)
            nc.tensor.matmul(ps_g2[:], ws_sb[:], h_sb[:, 512:1024], start=True, stop=True)
            g_row = sbp.tile([1, IPC], dt.float32)
            nc.vector.tensor_copy(g_row[:, 0:512], ps_g1[:])
            nc.vector.tensor_copy(g_row[:, 512:1024], ps_g2[:])

            g_bounce = drp.tile([1, IPC], dt.float32)
            g_all = drp.tile([NCORES, IPC], dt.float32)
            nc.gpsimd.dma_start(out=g_bounce[:], in_=g_row[:])
            nc.gpsimd.collective_compute(
                "AllGather",
                op.bypass,
                replica_groups=[list(range(NCORES))],
                ins=[g_bounce.opt()],
                outs=[g_all.opt()],
            )

            # ---- layer 2 prep ----
            # g in partition-major layout via PE transpose of [64,128] rows
            g_pm = sbp.tile([64, 128], dt.float32)
            nc.sync.dma_start(
                out=g_pm[:], in_=g_all[:].rearrange("c (q p) -> (c q) p", p=128)
            )
            ps_gt = psp.tile([128, 64], dt.float32, tag="g1")
            nc.tensor.transpose(ps_gt[:], g_pm[:], eye_sb[:])
            c2_sb = sbp.tile([128, JC], dt.float32)
            e2_sb = sbp.tile([128, JC], dt.float32)
            gpm_sb = sbp.tile([128, JC], dt.float32)
            nc.scalar.activation(
                c2_sb[:], ps_gt[:], AF.Exp, bias=0.8 * f2_bo, scale=0.8 * f2_wo
            )
            nc.scalar.activation(
                e2_sb[:], ps_gt[:], AF.Exp, bias=0.2 * f2_bo, scale=0.2 * f2_wo
            )
            nc.vector.tensor_copy(gpm_sb[:], ps_gt[:])
            # layer-2 stationary operand: col 0 = g*b2', col 32 = b2' (rowsum);
            # 33-wide so the PSUM rowsum row sits at partition 32 (aligned base)
            L2W = 33
            fa22 = sbp.tile([128, JC * L2W], dt.bfloat16)
            nc.vector.memset(fa22[:], 0.0)
            f22v = fa22[:].rearrange("p (q t) -> p q t", t=L2W)
            nc.vector.tensor_tensor(f22v[:, :, 0], gpm_sb[:], e2_sb[:], op.mult)
            nc.vector.tensor_copy(f22v[:, :, 32], e2_sb[:])

            # d2 = exp(0.8*(f1_wo*g_local + f1_bo)) broadcast to 128 partitions
            d2row = sbp.tile([1, IPC], dt.float32)
            nc.scalar.activation(
                d2row[:], g_row[:], AF.Exp, bias=0.8 * f1_bo, scale=0.8 * f1_wo
            )
            ps_d2a = psp.tile([128, 512], dt.float32, tag="bc_a")
            ps_d2b = psp.tile([128, 512], dt.float32, tag="bc_b")
            nc.tensor.matmul(ps_d2a[:], ones_sb[:], d2row[:, 0:512], start=True, stop=True)
            nc.tensor.matmul(ps_d2b[:], ones_sb[:], d2row[:, 512:1024], start=True, stop=True)
            d2_sb = sbp.tile([128, IPC], dt.bfloat16)
            nc.vector.tensor_copy(d2_sb[:, 0:512], ps_d2a[:])
            nc.vector.tensor_copy(d2_sb[:, 512:1024], ps_d2b[:])

            # ---- layer 2 main loop ----
            touch2 = sbp.tile([128, 1], dt.float32, tag="touch")
            nc.vector.tensor_copy(touch2[:], d2_sb[:, 0:1])
            nc.vector.tensor_copy(touch2[:], c2_sb[:, 0:1])

            ps2a = psp.tile([L2W, 512], dt.float32, tag="acc_a")
            ps2b = psp.tile([L2W, 512], dt.float32, tag="acc_b")
            for jc in range(JC):
                v = vwp.tile([128, IPC], dt.bfloat16, tag="v")
                ts_eng.tensor_scalar(
                    v[:], d2_sb[:], c2_sb[:, jc : jc + 1], 1.0, op.mult, op.max
                )
                w = vwp.tile([128, IPC], dt.bfloat16, tag="w")
                nc.vector.tensor_tensor(
                    w[:], v[:], adjT[:, jc * IPC : (jc + 1) * IPC], op.mult
                )
                lhsT2 = fa22[:, jc * L2W : (jc + 1) * L2W]
                nc.tensor.matmul(
                    ps2a[:], lhsT2, w[:, 0:512], start=(jc == 0), stop=(jc == JC - 1)
                )
                nc.tensor.matmul(
                    ps2b[:], lhsT2, w[:, 512:1024], start=(jc == 0), stop=(jc == JC - 1)
                )

            # ---- layer 2 epilogue: logits = vals/rowsum + vbiaso ----
            rc2a = sbp.tile([1, 512], dt.float32, tag="rc_a")
            rc2b = sbp.tile([1, 512], dt.float32, tag="rc_b")
            nc.vector.reciprocal(rc2a[:], ps2a[32:33, :])
            nc.vector.reciprocal(rc2b[:], ps2b[32:33, :])
            lg = sbp.tile([1, IPC], dt.float32)
            nc.vector.tensor_tensor(lg[:, 0:512], ps2a[0:1, :], rc2a[:], op.mult)
            nc.vector.tensor_tensor(lg[:, 512:1024], ps2b[0:1, :], rc2b[:], op.mult)
            nc.vector.tensor_tensor(lg[:], lg[:], vbo_sb[:], op.add)
            nc.sync.dma_start(out=lg_out[:, :], in_=lg[:])

    nc.finalize()
    return nc


def kernel(
    conv_feats,
    labels,
    node_byxs,
    adj,
    w_fts1,
    f1_w1,
    f1_b1,
    f2_w1,
    f2_b1,
    vbias1,
    w_ftso,
    f1_wo,
    f1_bo,
    f2_wo,
    f2_bo,
    vbiaso,
):
    global LAST_RESULT
    conv_feats = np.asarray(conv_feats, dtype=np.float32)
    labels = np.asarray(labels)
    node_byxs = np.asarray(node_byxs)
    adj = np.asarray(adj, dtype=np.float32)
    w_fts1 = np.asarray(w_fts1, dtype=np.float32)
    f1_w1 = np.asarray(f1_w1, dtype=np.float32)
    f1_b1 = np.asarray(f1_b1, dtype=np.float32)
    f2_w1 = np.asarray(f2_w1, dtype=np.float32)
    f2_b1 = np.asarray(f2_b1, dtype=np.float32)
    vbias1 = np.asarray(vbias1, dtype=np.float32)
    w_ftso = np.asarray(w_ftso, dtype=np.float32)
    vbiaso = np.asarray(vbiaso, dtype=np.float32)

    ys, xs = node_byxs[:, 1], node_byxs[:, 2]
    node_feats = conv_feats[0].transpose(1, 2, 0)[ys, xs]  # [N, CIN]
    node_labels = labels[0, ys, xs].astype(np.float32)

    # layer-1 per-node projections (tiny; the N x N work stays on device)
    fts = node_feats @ w_fts1.T  # [N, HID]
    f1 = fts @ f1_w1[0] + f1_b1[0]  # [N]
    f2 = fts @ f2_w1[0] + f2_b1[0]  # [N]
    bp = np.exp(0.2 * f2)
    c = np.exp(0.8 * f2)
    d = np.exp(0.8 * f1)

    fa2_full = np.concatenate([fts * bp[:, None], bp[:, None]], axis=1)  # [N, 65]
    fa2_dev = np.ascontiguousarray(
        fa2_full.reshape(JC, 128, 65).transpose(1, 0, 2).reshape(128, JC * 65)
    ).astype(BF16)
    ctile = np.ascontiguousarray(c.reshape(JC, 128).T).astype(np.float32)
    wsum = w_ftso[0].reshape(HEADS, HID).sum(axis=0).reshape(HID, 1).astype(np.float32)
    vb1t = np.ascontiguousarray(vbias1[0].T)  # [HID, N]
    vbot = np.ascontiguousarray(vbiaso[0, :, 0][None, :])  # [1, N]
    eye = np.eye(64, dtype=np.float32)

    nc_prog = _build_program(
        float(f1_wo[0, 0]), float(f1_bo[0]), float(f2_wo[0, 0]), float(f2_bo[0])
    )

    in_maps = []
    for core in range(NCORES):
        i0, i1 = core * IPC, (core + 1) * IPC
        in_maps.append(
            {
                "adjt": np.ascontiguousarray(adj[i0:i1, :].T),  # [N, IPC] f32
                "fa2": fa2_dev,
                "ctile": ctile,
                "drep": np.ascontiguousarray(
                    np.broadcast_to(d[None, i0:i1], (128, IPC))
                ).astype(BF16),
                "wsum": wsum,
                "vb1t": np.ascontiguousarray(vb1t[:, i0:i1]),
                "vbot": np.ascontiguousarray(vbot[:, i0:i1]),
                "eye": eye,
            }
        )

    from concourse.bass_utils import run_bass_kernel_spmd

    global LAST_PROG, LAST_IN_MAPS
    LAST_PROG = nc_prog
    LAST_IN_MAPS = in_maps
    trace = bool(int(os.environ.get("GAT_TRACE", "0")))
    LAST_RESULT = run_bass_kernel_spmd(
        nc_prog, in_maps, core_ids=list(range(NCORES)), trace=trace
    )
    results = LAST_RESULT.results

    h_full = np.concatenate([r["h_out"] for r in results], axis=1)  # [64, N]
    node_logits = np.concatenate([r["lg_out"][0] for r in results])  # [N]
    gnn_final_feats = np.tile(h_full, (HEADS, 1))  # [512, N]

    return (
        node_logits.astype(np.float32),
        gnn_final_feats.astype(np.float32),
        node_byxs,
        node_labels,
    )


# revision 20
# speedup vs baseline: 5.5632x; 5.5632x over previous
"""GAT (2-layer graph-attention) Trainium2 kernel, SPMD over 8 NeuronCores.

Sharding: core c owns destination nodes i in [c*1024, (c+1)*1024).  It receives
its row-slab of adj pre-transposed on host (shape [8192 src, 1024 dst]) so the
source dimension lands on SBUF partitions, which is what the PE matmul
contracts over.  adj is binary, so the on-load f32->bf16 cast is exact.

Math: with s_ij = f1_i + f2_j,
    exp(leaky_relu(s, 0.2)) = e^{0.2 f1_i} e^{0.2 f2_j} * max(e^{0.8 f2_j} e^{0.8 f1_i}, 1)
The e^{0.2 f2_j} factor is folded into the fts matmul operand, the e^{0.2 f1_i}
factor cancels between numerator and softmax row-sum, so the per-element work
is one dual-op tensor_scalar (mult+max) and one masking tensor_tensor mult.
The matmul computes [fts*b' ; b'] so row-sums come out of the same accumulation.
Between the two attention layers only the scalar per-node projection g needs to
be exchanged: a 4KB-per-core AllGather.
"""

import os
import sys

sys.path.insert(0, "/opt/trn_rl_repo")

import numpy as np
import ml_dtypes

BF16 = ml_dtypes.bfloat16

N = 8192
NCORES = 8
IPC = N // NCORES  # 1024 destination nodes per core
HID = 64
HEADS = 8
JC = N // 128  # 64 source chunks of 128

LAST_RESULT = None  # BassKernelResults of the most recent launch (for test.py)
LAST_PROG = None  # Bass program of the most recent launch (for test.py bench)
LAST_IN_MAPS = None  # per-core input maps of the most recent launch


def _build_program(f1_wo, f1_bo, f2_wo, f2_bo):
    import concourse.bass as bass
    import concourse.bacc as bacc
    import concourse.mybir as mybir
    from concourse.tile import TileContext

    dt = mybir.dt
    op = mybir.AluOpType
    AF = mybir.ActivationFunctionType

    ts_engine = os.environ.get("GAT_TS_ENGINE", "gpsimd")

    # Bacc (not plain Bass): its compile passes split multi-semaphore waits
    # into event-semaphore chains, which the TT/TS instruction formats need.
    nc = bacc.Bacc(None, num_devices=NCORES)

    adjt_in = nc.dram_tensor("adjt", [N, IPC], dt.float32, kind="ExternalInput")
    fa2_in = nc.dram_tensor("fa2", [128, JC * 65], dt.bfloat16, kind="ExternalInput")
    ct_in = nc.dram_tensor("ctile", [128, JC], dt.float32, kind="ExternalInput")
    dr_in = nc.dram_tensor("drep", [128, IPC], dt.bfloat16, kind="ExternalInput")
    ws_in = nc.dram_tensor("wsum", [HID, 1], dt.float32, kind="ExternalInput")
    vb1_in = nc.dram_tensor("vb1t", [HID, IPC], dt.float32, kind="ExternalInput")
    vbo_in = nc.dram_tensor("vbot", [1, IPC], dt.float32, kind="ExternalInput")
    eye_in = nc.dram_tensor("eye", [64, 64], dt.float32, kind="ExternalInput")

    h_out = nc.dram_tensor("h_out", [HID, IPC], dt.float32, kind="ExternalOutput")
    lg_out = nc.dram_tensor("lg_out", [1, IPC], dt.float32, kind="ExternalOutput")

    ts_eng = {"gpsimd": nc.gpsimd, "vector": nc.vector}[ts_engine]

    with TileContext(nc) as tc:
        with (
            tc.tile_pool(name="big", bufs=1) as bigp,
            tc.tile_pool(name="sb", bufs=1) as sbp,
            tc.tile_pool(name="vw", bufs=3) as vwp,
            tc.tile_pool(name="ps", bufs=1, space="PSUM") as psp,
            tc.tile_pool(name="dram", bufs=1, space="DRAM") as drp,
        ):
            # ---- persistent SBUF state ----
            adjT = bigp.tile([128, JC * IPC], dt.bfloat16)  # 16 MB: adj^T, bf16
            fa2_sb = sbp.tile([128, JC * 65], dt.bfloat16)
            ct_sb = sbp.tile([128, JC], dt.float32)
            dr_sb = sbp.tile([128, IPC], dt.bfloat16)
            ws_sb = sbp.tile([HID, 1], dt.float32)
            vb1_sb = sbp.tile([HID, IPC], dt.float32)
            vbo_sb = sbp.tile([1, IPC], dt.float32)
            eye_sb = sbp.tile([64, 64], dt.float32)
            ones_sb = sbp.tile([1, 128], dt.float32)

            nc.sync.dma_start(out=fa2_sb[:], in_=fa2_in[:, :])
            nc.sync.dma_start(out=ct_sb[:], in_=ct_in[:, :])
            nc.sync.dma_start(out=dr_sb[:], in_=dr_in[:, :])
            nc.sync.dma_start(out=ws_sb[:], in_=ws_in[:, :])
            nc.sync.dma_start(out=vb1_sb[:], in_=vb1_in[:, :])
            nc.sync.dma_start(out=vbo_sb[:], in_=vbo_in[:, :])
            nc.sync.dma_start(out=eye_sb[:], in_=eye_in[:, :])
            nc.vector.memset(ones_sb[:], 1.0)

            reps = int(os.environ.get("GAT_REPS", "1"))
            for _rep in range(reps):
                _one_pass(
                    nc, tc, dt, op, AF, ts_eng,
                    bigp, sbp, vwp, psp, drp,
                    adjT, fa2_sb, ct_sb, dr_sb, ws_sb, vb1_sb, vbo_sb, eye_sb,
                    ones_sb, adjt_in, h_out, lg_out,
                    f1_wo, f1_bo, f2_wo, f2_bo,
                )

    nc.finalize()
    return nc


def _one_pass(
    nc, tc, dt, op, AF, ts_eng,
    bigp, sbp, vwp, psp, drp,
    adjT, fa2_sb, ct_sb, dr_sb, ws_sb, vb1_sb, vbo_sb, eye_sb,
    ones_sb, adjt_in, h_out, lg_out,
    f1_wo, f1_bo, f2_wo, f2_bo,
):
    if True:
        if True:
            # adj^T slab load: f32 HBM -> bf16 SBUF (SWDGE cast DMA), 8 x 4MB
            adjt_r = adjt_in[:, :].rearrange("(b q p) i -> b p q i", p=128, q=8)
            for b in range(8):
                dst = adjT[:, b * 8 * IPC : (b + 1) * 8 * IPC].rearrange(
                    "p (q i) -> p q i", q=8
                )
                nc.gpsimd.dma_start(out=dst, in_=adjt_r[b])

            # ---- layer 1 main loop ----
            # The TensorScalarPtr ISA slot only fits one sync wait, so absorb
            # the input-DMA dependencies into a multi-wait-capable op on the
            # same engine first.
            touch = sbp.tile([128, 1], dt.float32, tag="touch")
            nc.vector.tensor_copy(touch[:], dr_sb[:, 0:1])
            nc.vector.tensor_copy(touch[:], ct_sb[:, 0:1])

            ps_a = psp.tile([65, 512], dt.float32, tag="acc_a")
            ps_b = psp.tile([65, 512], dt.float32, tag="acc_b")
            for jc in range(JC):
                if jc % 8 == 0:
                    # absorb this adj chunk's (possibly split) DMA semaphores
                    nc.vector.tensor_copy(touch[:], adjT[:, jc * IPC : jc * IPC + 1])
                v = vwp.tile([128, IPC], dt.bfloat16, tag="v")
                ts_eng.tensor_scalar(
                    v[:], dr_sb[:], ct_sb[:, jc : jc + 1], 1.0, op.mult, op.max
                )
                w = vwp.tile([128, IPC], dt.bfloat16, tag="w")
                nc.vector.tensor_tensor(
                    w[:], v[:], adjT[:, jc * IPC : (jc + 1) * IPC], op.mult
                )
                lhsT = fa2_sb[:, jc * 65 : (jc + 1) * 65]
                nc.tensor.matmul(
                    ps_a[:], lhsT, w[:, 0:512], start=(jc == 0), stop=(jc == JC - 1)
                )
                nc.tensor.matmul(
                    ps_b[:], lhsT, w[:, 512:1024], start=(jc == 0), stop=(jc == JC - 1)
                )

            # ---- layer 1 epilogue: h = elu(vals/rowsum + vbias1) ----
            rc_a = sbp.tile([1, 512], dt.float32, tag="rc_a")
            rc_b = sbp.tile([1, 512], dt.float32, tag="rc_b")
            nc.vector.reciprocal(rc_a[:], ps_a[64:65, :])
            nc.vector.reciprocal(rc_b[:], ps_b[64:65, :])
            ps_ra = psp.tile([64, 512], dt.float32, tag="bc_a")
            ps_rb = psp.tile([64, 512], dt.float32, tag="bc_b")
            nc.tensor.matmul(ps_ra[:], ones_sb[:, 0:64], rc_a[:], start=True, stop=True)
            nc.tensor.matmul(ps_rb[:], ones_sb[:, 0:64], rc_b[:], start=True, stop=True)
            rcr = sbp.tile([64, IPC], dt.float32)
            nc.vector.tensor_copy(rcr[:, 0:512], ps_ra[:])
            nc.vector.tensor_copy(rcr[:, 512:1024], ps_rb[:])
            xx = sbp.tile([64, IPC], dt.float32)
            nc.vector.tensor_tensor(xx[:, 0:512], ps_a[0:64, :], rcr[:, 0:512], op.mult)
            nc.vector.tensor_tensor(xx[:, 512:1024], ps_b[0:64, :], rcr[:, 512:1024], op.mult)
            nc.vector.tensor_tensor(xx[:], xx[:], vb1_sb[:], op.add)
            r1 = sbp.tile([64, IPC], dt.float32)
            nc.vector.tensor_scalar_max(r1[:], xx[:], 0.0)
            mn = sbp.tile([64, IPC], dt.float32, tag="rcr")  # rcr is dead here
            nc.vector.tensor_scalar_min(mn[:], xx[:], 0.0)
            ex = sbp.tile([64, IPC], dt.float32, tag="xx")  # xx dead after r1/mn
            nc.scalar.activation(ex[:], mn[:], AF.Exp)
            h_sb = sbp.tile([64, IPC], dt.float32)
            nc.vector.tensor_tensor(h_sb[:], r1[:], ex[:], op.add)
            nc.vector.tensor_scalar_sub(h_sb[:], h_sb[:], 1.0)
            nc.sync.dma_start(out=h_out[:, :], in_=h_sb[:])

            # ---- g = wsum . h ; AllGather g across cores ----
            ps_g1 = psp.tile([1, 512], dt.float32, tag="g1")
            ps_g2 = psp.tile([1, 512], dt.float32, tag="g2")
            nc.tensor.matmul(ps_g1[:], ws_sb[:], h_sb[:, 0:512], start=True, stop=True)
            nc.tensor.matmul(ps_g2[:], ws_sb[:], h_sb[:, 512:1024], start=True, stop=True)
            g_row = sbp.tile([1, IPC], dt.float32)
            nc.vector.tensor_copy(g_row[:, 0:512], ps_g1[:])
            nc.vector.tensor_copy(g_row[:, 512:1024], ps_g2[:])

            g_bounce = drp.tile([1, IPC], dt.float32)
            g_all = drp.tile([NCORES, IPC], dt.float32)
            nc.gpsimd.dma_start(out=g_bounce[:], in_=g_row[:])
            nc.gpsimd.collective_compute(
                "AllGather",
                op.bypass,
                replica_groups=[list(range(NCORES))],
                ins=[g_bounce.opt()],
                outs=[g_all.opt()],
            )

            # ---- layer 2 prep ----
            # g in partition-major layout via PE transpose of [64,128] rows
            g_pm = sbp.tile([64, 128], dt.float32)
            nc.sync.dma_start(
                out=g_pm[:], in_=g_all[:].rearrange("c (q p) -> (c q) p", p=128)
            )
            ps_gt = psp.tile([128, 64], dt.float32, tag="g1")
            nc.tensor.transpose(ps_gt[:], g_pm[:], eye_sb[:])
            c2_sb = sbp.tile([128, JC], dt.float32)
            e2_sb = sbp.tile([128, JC], dt.float32)
            gpm_sb = sbp.tile([128, JC], dt.float32)
            nc.scalar.activation(
                c2_sb[:], ps_gt[:], AF.Exp, bias=0.8 * f2_bo, scale=0.8 * f2_wo
            )
            nc.scalar.activation(
                e2_sb[:], ps_gt[:], AF.Exp, bias=0.2 * f2_bo, scale=0.2 * f2_wo
            )
            nc.vector.tensor_copy(gpm_sb[:], ps_gt[:])
            # layer-2 stationary operand: col 0 = g*b2', col 32 = b2' (rowsum);
            # 33-wide so the PSUM rowsum row sits at partition 32 (aligned base)
            L2W = 33
            fa22 = sbp.tile([128, JC * L2W], dt.bfloat16)
            nc.vector.memset(fa22[:], 0.0)
            f22v = fa22[:].rearrange("p (q t) -> p q t", t=L2W)
            nc.vector.tensor_tensor(f22v[:, :, 0], gpm_sb[:], e2_sb[:], op.mult)
            nc.vector.tensor_copy(f22v[:, :, 32], e2_sb[:])

            # d2 = exp(0.8*(f1_wo*g_local + f1_bo)) broadcast to 128 partitions
            d2row = sbp.tile([1, IPC], dt.float32)
            nc.scalar.activation(
                d2row[:], g_row[:], AF.Exp, bias=0.8 * f1_bo, scale=0.8 * f1_wo
            )
            ps_d2a = psp.tile([128, 512], dt.float32, tag="bc_a")
            ps_d2b = psp.tile([128, 512], dt.float32, tag="bc_b")
            nc.tensor.matmul(ps_d2a[:], ones_sb[:], d2row[:, 0:512], start=True, stop=True)
            nc.tensor.matmul(ps_d2b[:], ones_sb[:], d2row[:, 512:1024], start=True, stop=True)
            d2_sb = sbp.tile([128, IPC], dt.bfloat16)
            nc.vector.tensor_copy(d2_sb[:, 0:512], ps_d2a[:])
            nc.vector.tensor_copy(d2_sb[:, 512:1024], ps_d2b[:])

            # ---- layer 2 main loop ----
            touch2 = sbp.tile([128, 1], dt.float32, tag="touch")
            nc.vector.tensor_copy(touch2[:], d2_sb[:, 0:1])
            nc.vector.tensor_copy(touch2[:], c2_sb[:, 0:1])

            ps2a = psp.tile([L2W, 512], dt.float32, tag="acc_a")
            ps2b = psp.tile([L2W, 512], dt.float32, tag="acc_b")
            for jc in range(JC):
                v = vwp.tile([128, IPC], dt.bfloat16, tag="v")
                ts_eng.tensor_scalar(
                    v[:], d2_sb[:], c2_sb[:, jc : jc + 1], 1.0, op.mult, op.max
                )
                w = vwp.tile([128, IPC], dt.bfloat16, tag="w")
                nc.vector.tensor_tensor(
                    w[:], v[:], adjT[:, jc * IPC : (jc + 1) * IPC], op.mult
                )
                lhsT2 = fa22[:, jc * L2W : (jc + 1) * L2W]
                nc.tensor.matmul(
                    ps2a[:], lhsT2, w[:, 0:512], start=(jc == 0), stop=(jc == JC - 1)
                )
                nc.tensor.matmul(
                    ps2b[:], lhsT2, w[:, 512:1024], start=(jc == 0), stop=(jc == JC - 1)
                )

            # ---- layer 2 epilogue: logits = vals/rowsum + vbiaso ----
            rc2a = sbp.tile([1, 512], dt.float32, tag="rc_a")
            rc2b = sbp.tile([1, 512], dt.float32, tag="rc_b")
            nc.vector.reciprocal(rc2a[:], ps2a[32:33, :])
            nc.vector.reciprocal(rc2b[:], ps2b[32:33, :])
            lg = sbp.tile([1, IPC], dt.float32)
            nc.vector.tensor_tensor(lg[:, 0:512], ps2a[0:1, :], rc2a[:], op.mult)
            nc.vector.tensor_tensor(lg[:, 512:1024], ps2b[0:1, :], rc2b[:], op.mult)
            nc.vector.tensor_tensor(lg[:], lg[:], vbo_sb[:], op.add)
            nc.sync.dma_start(out=lg_out[:, :], in_=lg[:])


def kernel(
    conv_feats,
    labels,
    node_byxs,
    adj,
    w_fts1,
    f1_w1,
    f1_b1,
    f2_w1,
    f2_b1,
    vbias1,
    w_ftso,
    f1_wo,
    f1_bo,
    f2_wo,
    f2_bo,
    vbiaso,
):
    global LAST_RESULT
    conv_feats = np.asarray(conv_feats, dtype=np.float32)
    labels = np.asarray(labels)
    node_byxs = np.asarray(node_byxs)
    adj = np.asarray(adj, dtype=np.float32)
    w_fts1 = np.asarray(w_fts1, dtype=np.float32)
    f1_w1 = np.asarray(f1_w1, dtype=np.float32)
    f1_b1 = np.asarray(f1_b1, dtype=np.float32)
    f2_w1 = np.asarray(f2_w1, dtype=np.float32)
    f2_b1 = np.asarray(f2_b1, dtype=np.float32)
    vbias1 = np.asarray(vbias1, dtype=np.float32)
    w_ftso = np.asarray(w_ftso, dtype=np.float32)
    vbiaso = np.asarray(vbiaso, dtype=np.float32)

    ys, xs = node_byxs[:, 1], node_byxs[:, 2]
    node_feats = conv_feats[0].transpose(1, 2, 0)[ys, xs]  # [N, CIN]
    node_labels = labels[0, ys, xs].astype(np.float32)

    # layer-1 per-node projections (tiny; the N x N work stays on device)
    fts = node_feats @ w_fts1.T  # [N, HID]
    f1 = fts @ f1_w1[0] + f1_b1[0]  # [N]
    f2 = fts @ f2_w1[0] + f2_b1[0]  # [N]
    bp = np.exp(0.2 * f2)
    c = np.exp(0.8 * f2)
    d = np.exp(0.8 * f1)

    fa2_full = np.concatenate([fts * bp[:, None], bp[:, None]], axis=1)  # [N, 65]
    fa2_dev = np.ascontiguousarray(
        fa2_full.reshape(JC, 128, 65).transpose(1, 0, 2).reshape(128, JC * 65)
    ).astype(BF16)
    ctile = np.ascontiguousarray(c.reshape(JC, 128).T).astype(np.float32)
    wsum = w_ftso[0].reshape(HEADS, HID).sum(axis=0).reshape(HID, 1).astype(np.float32)
    vb1t = np.ascontiguousarray(vbias1[0].T)  # [HID, N]
    vbot = np.ascontiguousarray(vbiaso[0, :, 0][None, :])  # [1, N]
    eye = np.eye(64, dtype=np.float32)

    nc_prog = _build_program(
        float(f1_wo[0, 0]), float(f1_bo[0]), float(f2_wo[0, 0]), float(f2_bo[0])
    )

    in_maps = []
    for core in range(NCORES):
        i0, i1 = core * IPC, (core + 1) * IPC
        in_maps.append(
            {
                "adjt": np.ascontiguousarray(adj[i0:i1, :].T),  # [N, IPC] f32
                "fa2": fa2_dev,
                "ctile": ctile,
                "drep": np.ascontiguousarray(
                    np.broadcast_to(d[None, i0:i1], (128, IPC))
                ).astype(BF16),
                "wsum": wsum,
                "vb1t": np.ascontiguousarray(vb1t[:, i0:i1]),
                "vbot": np.ascontiguousarray(vbot[:, i0:i1]),
                "eye": eye,
            }
        )

    from concourse.bass_utils import run_bass_kernel_spmd

    global LAST_PROG, LAST_IN_MAPS
    LAST_PROG = nc_prog
    LAST_IN_MAPS = in_maps
    trace = bool(int(os.environ.get("GAT_TRACE", "0")))
    LAST_RESULT = run_bass_kernel_spmd(
        nc_prog, in_maps, core_ids=list(range(NCORES)), trace=trace
    )
    results = LAST_RESULT.results

    h_full = np.concatenate([r["h_out"] for r in results], axis=1)  # [64, N]
    node_logits = np.concatenate([r["lg_out"][0] for r in results])  # [N]
    gnn_final_feats = np.tile(h_full, (HEADS, 1))  # [512, N]

    return (
        node_logits.astype(np.float32),
        gnn_final_feats.astype(np.float32),
        node_byxs,
        node_labels,
    )


# revision 21
# speedup vs baseline: 223.0500x; 40.0937x over previous
"""GAT (2-layer graph-attention) Trainium2 kernel, SPMD over 8 NeuronCores.

Sharding: core c owns destination nodes i in [c*1024, (c+1)*1024).  It receives
its row-slab of adj pre-transposed on host (shape [8192 src, 1024 dst]) so the
source dimension lands on SBUF partitions, which is what the PE matmul
contracts over.  adj is binary, so the on-load f32->bf16 cast is exact.

Math: with s_ij = f1_i + f2_j,
    exp(leaky_relu(s, 0.2)) = e^{0.2 f1_i} e^{0.2 f2_j} * max(e^{0.8 f2_j} e^{0.8 f1_i}, 1)
The e^{0.2 f2_j} factor is folded into the fts matmul operand, the e^{0.2 f1_i}
factor cancels between numerator and softmax row-sum, so the per-element work
is one dual-op tensor_scalar (mult+max) and one masking tensor_tensor mult.
The matmul computes [fts*b' ; b'] so row-sums come out of the same accumulation.
Between the two attention layers only the scalar per-node projection g needs to
be exchanged: a 4KB-per-core AllGather.
"""

import os
import sys

sys.path.insert(0, "/opt/trn_rl_repo")

import numpy as np
import ml_dtypes

BF16 = ml_dtypes.bfloat16

N = 8192
NCORES = 8
IPC = N // NCORES  # 1024 destination nodes per core
HID = 64
HEADS = 8
JC = N // 128  # 64 source chunks of 128

LAST_RESULT = None  # BassKernelResults of the most recent launch (for test.py)
LAST_PROG = None  # Bass program of the most recent launch (for test.py bench)
LAST_IN_MAPS = None  # per-core input maps of the most recent launch


def _build_program(f1_wo, f1_bo, f2_wo, f2_bo):
    import concourse.bass as bass
    import concourse.bacc as bacc
    import concourse.mybir as mybir
    from concourse.tile import TileContext

    dt = mybir.dt
    op = mybir.AluOpType
    AF = mybir.ActivationFunctionType

    ts_engine = os.environ.get("GAT_TS_ENGINE", "vector")

    # Bacc (not plain Bass): its compile passes split multi-semaphore waits
    # into event-semaphore chains, which the TT/TS instruction formats need.
    nc = bacc.Bacc(None, num_devices=NCORES)

    adjt_in = nc.dram_tensor("adjt", [N, IPC], dt.float32, kind="ExternalInput")
    fa2_in = nc.dram_tensor("fa2", [128, JC * 65], dt.bfloat16, kind="ExternalInput")
    ct_in = nc.dram_tensor("ctile", [128, JC], dt.float32, kind="ExternalInput")
    dr_in = nc.dram_tensor("drep", [128, IPC], dt.bfloat16, kind="ExternalInput")
    ws_in = nc.dram_tensor("wsum", [HID, 1], dt.float32, kind="ExternalInput")
    vb1_in = nc.dram_tensor("vb1t", [HID, IPC], dt.float32, kind="ExternalInput")
    vbo_in = nc.dram_tensor("vbot", [1, IPC], dt.float32, kind="ExternalInput")
    eye_in = nc.dram_tensor("eye", [64, 64], dt.float32, kind="ExternalInput")

    h_out = nc.dram_tensor("h_out", [HID, IPC], dt.float32, kind="ExternalOutput")
    lg_out = nc.dram_tensor("lg_out", [1, IPC], dt.float32, kind="ExternalOutput")

    ts_eng = {"gpsimd": nc.gpsimd, "vector": nc.vector}[ts_engine]

    with TileContext(nc) as tc:
        with (
            tc.tile_pool(name="big", bufs=1) as bigp,
            tc.tile_pool(name="sb", bufs=1) as sbp,
            tc.tile_pool(name="vw", bufs=3) as vwp,
            tc.tile_pool(name="ps", bufs=1, space="PSUM") as psp,
            tc.tile_pool(name="dram", bufs=1, space="DRAM") as drp,
        ):
            # ---- persistent SBUF state ----
            adjT = bigp.tile([128, JC * IPC], dt.bfloat16)  # 16 MB: adj^T, bf16
            fa2_sb = sbp.tile([128, JC * 65], dt.bfloat16)
            ct_sb = sbp.tile([128, JC], dt.float32)
            dr_sb = sbp.tile([128, IPC], dt.bfloat16)
            ws_sb = sbp.tile([HID, 1], dt.float32)
            vb1_sb = sbp.tile([HID, IPC], dt.float32)
            vbo_sb = sbp.tile([1, IPC], dt.float32)
            eye_sb = sbp.tile([64, 64], dt.float32)
            ones_sb = sbp.tile([1, 128], dt.float32)

            nc.sync.dma_start(out=fa2_sb[:], in_=fa2_in[:, :])
            nc.sync.dma_start(out=ct_sb[:], in_=ct_in[:, :])
            nc.sync.dma_start(out=dr_sb[:], in_=dr_in[:, :])
            nc.sync.dma_start(out=ws_sb[:], in_=ws_in[:, :])
            nc.sync.dma_start(out=vb1_sb[:], in_=vb1_in[:, :])
            nc.sync.dma_start(out=vbo_sb[:], in_=vbo_in[:, :])
            nc.sync.dma_start(out=eye_sb[:], in_=eye_in[:, :])
            nc.vector.memset(ones_sb[:], 1.0)

            reps = int(os.environ.get("GAT_REPS", "1"))
            for _rep in range(reps):
                _one_pass(
                    nc, tc, dt, op, AF, ts_eng,
                    bigp, sbp, vwp, psp, drp,
                    adjT, fa2_sb, ct_sb, dr_sb, ws_sb, vb1_sb, vbo_sb, eye_sb,
                    ones_sb, adjt_in, h_out, lg_out,
                    f1_wo, f1_bo, f2_wo, f2_bo,
                )

    nc.finalize()
    return nc


def _one_pass(
    nc, tc, dt, op, AF, ts_eng,
    bigp, sbp, vwp, psp, drp,
    adjT, fa2_sb, ct_sb, dr_sb, ws_sb, vb1_sb, vbo_sb, eye_sb,
    ones_sb, adjt_in, h_out, lg_out,
    f1_wo, f1_bo, f2_wo, f2_bo,
):
    if True:
        if True:
            # adj^T slab load: f32 HBM -> bf16 SBUF (SWDGE cast DMA), 8 x 4MB
            adjt_r = adjt_in[:, :].rearrange("(b q p) i -> b p q i", p=128, q=8)
            for b in range(8):
                dst = adjT[:, b * 8 * IPC : (b + 1) * 8 * IPC].rearrange(
                    "p (q i) -> p q i", q=8
                )
                nc.gpsimd.dma_start(out=dst, in_=adjt_r[b])

            # ---- layer 1 main loop ----
            # The TensorScalarPtr ISA slot only fits one sync wait, so absorb
            # the input-DMA dependencies into a multi-wait-capable op on the
            # same engine first.
            touch = sbp.tile([128, 1], dt.float32, tag="touch")
            nc.vector.tensor_copy(touch[:], dr_sb[:, 0:1])
            nc.vector.tensor_copy(touch[:], ct_sb[:, 0:1])

            ps_a = psp.tile([65, 512], dt.float32, tag="acc_a")
            ps_b = psp.tile([65, 512], dt.float32, tag="acc_b")
            for jc in range(JC):
                if jc % 8 == 0:
                    # absorb this adj chunk's (possibly split) DMA semaphores
                    nc.vector.tensor_copy(touch[:], adjT[:, jc * IPC : jc * IPC + 1])
                v = vwp.tile([128, IPC], dt.bfloat16, tag="v")
                ts_eng.tensor_scalar(
                    v[:], dr_sb[:], ct_sb[:, jc : jc + 1], 1.0, op.mult, op.max
                )
                w = vwp.tile([128, IPC], dt.bfloat16, tag="w")
                nc.vector.tensor_tensor(
                    w[:], v[:], adjT[:, jc * IPC : (jc + 1) * IPC], op.mult
                )
                lhsT = fa2_sb[:, jc * 65 : (jc + 1) * 65]
                nc.tensor.matmul(
                    ps_a[:], lhsT, w[:, 0:512], start=(jc == 0), stop=(jc == JC - 1)
                )
                nc.tensor.matmul(
                    ps_b[:], lhsT, w[:, 512:1024], start=(jc == 0), stop=(jc == JC - 1)
                )

            # ---- layer 1 epilogue: h = elu(vals/rowsum + vbias1) ----
            rc_a = sbp.tile([1, 512], dt.float32, tag="rc_a")
            rc_b = sbp.tile([1, 512], dt.float32, tag="rc_b")
            nc.vector.reciprocal(rc_a[:], ps_a[64:65, :])
            nc.vector.reciprocal(rc_b[:], ps_b[64:65, :])
            ps_ra = psp.tile([64, 512], dt.float32, tag="bc_a")
            ps_rb = psp.tile([64, 512], dt.float32, tag="bc_b")
            nc.tensor.matmul(ps_ra[:], ones_sb[:, 0:64], rc_a[:], start=True, stop=True)
            nc.tensor.matmul(ps_rb[:], ones_sb[:, 0:64], rc_b[:], start=True, stop=True)
            rcr = sbp.tile([64, IPC], dt.float32)
            nc.vector.tensor_copy(rcr[:, 0:512], ps_ra[:])
            nc.vector.tensor_copy(rcr[:, 512:1024], ps_rb[:])
            xx = sbp.tile([64, IPC], dt.float32)
            nc.vector.tensor_tensor(xx[:, 0:512], ps_a[0:64, :], rcr[:, 0:512], op.mult)
            nc.vector.tensor_tensor(xx[:, 512:1024], ps_b[0:64, :], rcr[:, 512:1024], op.mult)
            nc.vector.tensor_tensor(xx[:], xx[:], vb1_sb[:], op.add)
            r1 = sbp.tile([64, IPC], dt.float32)
            nc.vector.tensor_scalar_max(r1[:], xx[:], 0.0)
            mn = sbp.tile([64, IPC], dt.float32, tag="rcr")  # rcr is dead here
            nc.vector.tensor_scalar_min(mn[:], xx[:], 0.0)
            ex = sbp.tile([64, IPC], dt.float32, tag="xx")  # xx dead after r1/mn
            nc.scalar.activation(ex[:], mn[:], AF.Exp)
            h_sb = sbp.tile([64, IPC], dt.float32)
            nc.vector.tensor_tensor(h_sb[:], r1[:], ex[:], op.add)
            nc.vector.tensor_scalar_sub(h_sb[:], h_sb[:], 1.0)
            nc.sync.dma_start(out=h_out[:, :], in_=h_sb[:])

            # ---- g = wsum . h ; AllGather g across cores ----
            ps_g1 = psp.tile([1, 512], dt.float32, tag="g1")
            ps_g2 = psp.tile([1, 512], dt.float32, tag="g2")
            nc.tensor.matmul(ps_g1[:], ws_sb[:], h_sb[:, 0:512], start=True, stop=True)
            nc.tensor.matmul(ps_g2[:], ws_sb[:], h_sb[:, 512:1024], start=True, stop=True)
            g_row = sbp.tile([1, IPC], dt.float32)
            nc.vector.tensor_copy(g_row[:, 0:512], ps_g1[:])
            nc.vector.tensor_copy(g_row[:, 512:1024], ps_g2[:])

            g_bounce = drp.tile([1, IPC], dt.float32)
            g_all = drp.tile([NCORES, IPC], dt.float32)
            nc.gpsimd.dma_start(out=g_bounce[:], in_=g_row[:])
            nc.gpsimd.collective_compute(
                "AllGather",
                op.bypass,
                replica_groups=[list(range(NCORES))],
                ins=[g_bounce.opt()],
                outs=[g_all.opt()],
            )

            # ---- layer 2 prep ----
            # g in partition-major layout via PE transpose of [64,128] rows
            g_pm = sbp.tile([64, 128], dt.float32)
            nc.sync.dma_start(
                out=g_pm[:], in_=g_all[:].rearrange("c (q p) -> (c q) p", p=128)
            )
            ps_gt = psp.tile([128, 64], dt.float32, tag="g1")
            nc.tensor.transpose(ps_gt[:], g_pm[:], eye_sb[:])
            c2_sb = sbp.tile([128, JC], dt.float32)
            e2_sb = sbp.tile([128, JC], dt.float32)
            gpm_sb = sbp.tile([128, JC], dt.float32)
            nc.scalar.activation(
                c2_sb[:], ps_gt[:], AF.Exp, bias=0.8 * f2_bo, scale=0.8 * f2_wo
            )
            nc.scalar.activation(
                e2_sb[:], ps_gt[:], AF.Exp, bias=0.2 * f2_bo, scale=0.2 * f2_wo
            )
            nc.vector.tensor_copy(gpm_sb[:], ps_gt[:])
            # layer-2 stationary operand: col 0 = g*b2', col 32 = b2' (rowsum);
            # 33-wide so the PSUM rowsum row sits at partition 32 (aligned base)
            L2W = 33
            fa22 = sbp.tile([128, JC * L2W], dt.bfloat16)
            nc.vector.memset(fa22[:], 0.0)
            f22v = fa22[:].rearrange("p (q t) -> p q t", t=L2W)
            nc.vector.tensor_tensor(f22v[:, :, 0], gpm_sb[:], e2_sb[:], op.mult)
            nc.vector.tensor_copy(f22v[:, :, 32], e2_sb[:])

            # d2 = exp(0.8*(f1_wo*g_local + f1_bo)) broadcast to 128 partitions
            d2row = sbp.tile([1, IPC], dt.float32)
            nc.scalar.activation(
                d2row[:], g_row[:], AF.Exp, bias=0.8 * f1_bo, scale=0.8 * f1_wo
            )
            ps_d2a = psp.tile([128, 512], dt.float32, tag="bc_a")
            ps_d2b = psp.tile([128, 512], dt.float32, tag="bc_b")
            nc.tensor.matmul(ps_d2a[:], ones_sb[:], d2row[:, 0:512], start=True, stop=True)
            nc.tensor.matmul(ps_d2b[:], ones_sb[:], d2row[:, 512:1024], start=True, stop=True)
            d2_sb = sbp.tile([128, IPC], dt.bfloat16)
            nc.vector.tensor_copy(d2_sb[:, 0:512], ps_d2a[:])
            nc.vector.tensor_copy(d2_sb[:, 512:1024], ps_d2b[:])

            # ---- layer 2 main loop ----
            touch2 = sbp.tile([128, 1], dt.float32, tag="touch")
            nc.vector.tensor_copy(touch2[:], d2_sb[:, 0:1])
            nc.vector.tensor_copy(touch2[:], c2_sb[:, 0:1])

            ps2a = psp.tile([L2W, 512], dt.float32, tag="acc_a")
            ps2b = psp.tile([L2W, 512], dt.float32, tag="acc_b")
            for jc in range(JC):
                v = vwp.tile([128, IPC], dt.bfloat16, tag="v")
                ts_eng.tensor_scalar(
                    v[:], d2_sb[:], c2_sb[:, jc : jc + 1], 1.0, op.mult, op.max
                )
                w = vwp.tile([128, IPC], dt.bfloat16, tag="w")
                nc.vector.tensor_tensor(
                    w[:], v[:], adjT[:, jc * IPC : (jc + 1) * IPC], op.mult
                )
                lhsT2 = fa22[:, jc * L2W : (jc + 1) * L2W]
                nc.tensor.matmul(
                    ps2a[:], lhsT2, w[:, 0:512], start=(jc == 0), stop=(jc == JC - 1)
                )
                nc.tensor.matmul(
                    ps2b[:], lhsT2, w[:, 512:1024], start=(jc == 0), stop=(jc == JC - 1)
                )

            # ---- layer 2 epilogue: logits = vals/rowsum + vbiaso ----
            rc2a = sbp.tile([1, 512], dt.float32, tag="rc_a")
            rc2b = sbp.tile([1, 512], dt.float32, tag="rc_b")
            nc.vector.reciprocal(rc2a[:], ps2a[32:33, :])
            nc.vector.reciprocal(rc2b[:], ps2b[32:33, :])
            lg = sbp.tile([1, IPC], dt.float32)
            nc.vector.tensor_tensor(lg[:, 0:512], ps2a[0:1, :], rc2a[:], op.mult)
            nc.vector.tensor_tensor(lg[:, 512:1024], ps2b[0:1, :], rc2b[:], op.mult)
            nc.vector.tensor_tensor(lg[:], lg[:], vbo_sb[:], op.add)
            nc.sync.dma_start(out=lg_out[:, :], in_=lg[:])


def kernel(
    conv_feats,
    labels,
    node_byxs,
    adj,
    w_fts1,
    f1_w1,
    f1_b1,
    f2_w1,
    f2_b1,
    vbias1,
    w_ftso,
    f1_wo,
    f1_bo,
    f2_wo,
    f2_bo,
    vbiaso,
):
    global LAST_RESULT
    conv_feats = np.asarray(conv_feats, dtype=np.float32)
    labels = np.asarray(labels)
    node_byxs = np.asarray(node_byxs)
    adj = np.asarray(adj, dtype=np.float32)
    w_fts1 = np.asarray(w_fts1, dtype=np.float32)
    f1_w1 = np.asarray(f1_w1, dtype=np.float32)
    f1_b1 = np.asarray(f1_b1, dtype=np.float32)
    f2_w1 = np.asarray(f2_w1, dtype=np.float32)
    f2_b1 = np.asarray(f2_b1, dtype=np.float32)
    vbias1 = np.asarray(vbias1, dtype=np.float32)
    w_ftso = np.asarray(w_ftso, dtype=np.float32)
    vbiaso = np.asarray(vbiaso, dtype=np.float32)

    ys, xs = node_byxs[:, 1], node_byxs[:, 2]
    node_feats = conv_feats[0].transpose(1, 2, 0)[ys, xs]  # [N, CIN]
    node_labels = labels[0, ys, xs].astype(np.float32)

    # layer-1 per-node projections (tiny; the N x N work stays on device)
    fts = node_feats @ w_fts1.T  # [N, HID]
    f1 = fts @ f1_w1[0] + f1_b1[0]  # [N]
    f2 = fts @ f2_w1[0] + f2_b1[0]  # [N]
    bp = np.exp(0.2 * f2)
    c = np.exp(0.8 * f2)
    d = np.exp(0.8 * f1)

    fa2_full = np.concatenate([fts * bp[:, None], bp[:, None]], axis=1)  # [N, 65]
    fa2_dev = np.ascontiguousarray(
        fa2_full.reshape(JC, 128, 65).transpose(1, 0, 2).reshape(128, JC * 65)
    ).astype(BF16)
    ctile = np.ascontiguousarray(c.reshape(JC, 128).T).astype(np.float32)
    wsum = w_ftso[0].reshape(HEADS, HID).sum(axis=0).reshape(HID, 1).astype(np.float32)
    vb1t = np.ascontiguousarray(vbias1[0].T)  # [HID, N]
    vbot = np.ascontiguousarray(vbiaso[0, :, 0][None, :])  # [1, N]
    eye = np.eye(64, dtype=np.float32)

    nc_prog = _build_program(
        float(f1_wo[0, 0]), float(f1_bo[0]), float(f2_wo[0, 0]), float(f2_bo[0])
    )

    in_maps = []
    for core in range(NCORES):
        i0, i1 = core * IPC, (core + 1) * IPC
        in_maps.append(
            {
                "adjt": np.ascontiguousarray(adj[i0:i1, :].T),  # [N, IPC] f32
                "fa2": fa2_dev,
                "ctile": ctile,
                "drep": np.ascontiguousarray(
                    np.broadcast_to(d[None, i0:i1], (128, IPC))
                ).astype(BF16),
                "wsum": wsum,
                "vb1t": np.ascontiguousarray(vb1t[:, i0:i1]),
                "vbot": np.ascontiguousarray(vbot[:, i0:i1]),
                "eye": eye,
            }
        )

    from concourse.bass_utils import run_bass_kernel_spmd

    global LAST_PROG, LAST_IN_MAPS
    LAST_PROG = nc_prog
    LAST_IN_MAPS = in_maps
    trace = bool(int(os.environ.get("GAT_TRACE", "0")))
    LAST_RESULT = run_bass_kernel_spmd(
        nc_prog, in_maps, core_ids=list(range(NCORES)), trace=trace
    )
    results = LAST_RESULT.results

    h_full = np.concatenate([r["h_out"] for r in results], axis=1)  # [64, N]
    node_logits = np.concatenate([r["lg_out"][0] for r in results])  # [N]
    gnn_final_feats = np.tile(h_full, (HEADS, 1))  # [512, N]

    return (
        node_logits.astype(np.float32),
        gnn_final_feats.astype(np.float32),
        node_byxs,
        node_labels,
    )
